# revision 21
# baseline (speedup 1.0000x reference)
"""HTSK fuzzy-system kernel for Trainium2 (Bass/Tile), 8-core data-parallel.

Math (per batch row b):
  S     = H/sigma^2 + EPS                          (D,R)
  m     = mean_d(-(X_bd - C_dr)^2 * S_dr)          (B,R)
        = X^2 @ (-S/D) + X @ (2*S*C/D) + K2        (matmul expansion)
  e     = exp(m)            (m <= 0 always, so no max-subtraction needed)
  out   = (1/sum_r e) * ( sum_r e_br * G_bro  +  e @ (W2 + 1 b^T) )
  G     = X @ Wt,  Wt[d, o*R + r] = W[r*D+d, o]    (B, O*R)  "(o,r)" col layout

The (o,r) column layout makes the e-broadcast multiply a stride-1-innermost
DVE op (2x mode) and the r-reduction a halving tree over the innermost axis.

Sharding: batch B=4096 split 512 rows per core; weights replicated.
"""
import sys
import types
from contextlib import ExitStack

import numpy as np

sys.path.insert(0, "/opt/trn_rl_repo")

# NTFF profile-hook registry: trn_boot sets it at jax init, concourse
# bass_utils reads it when trace=True. The container's antenv package lacks
# this submodule, so provide it before anything imports jax/concourse.
if "antenv.axon_hooks" not in sys.modules:
    _ah = types.ModuleType("antenv.axon_hooks")
    _ah._hook = None

    def _set_hook(hook):
        _ah._hook = hook

    def _get_hook():
        return _ah._hook

    _ah.set_axon_ntff_profile_hook = _set_hook
    _ah.get_axon_ntff_profile_hook = _get_hook
    sys.modules["antenv.axon_hooks"] = _ah

import ml_dtypes  # noqa: E402
import concourse.bass as bass  # noqa: E402
import concourse.bacc as bacc  # noqa: E402
import concourse.tile as tile  # noqa: E402
from concourse import mybir  # noqa: E402
from concourse import bass_utils  # noqa: E402
from concourse.masks import make_identity  # noqa: E402

H = 0.5
EPS = 1e-8
B, D, R, O = 4096, 256, 128, 64
NCORES = 8
BL = B // NCORES          # 512 batch rows per core
NT = BL // 128            # 4 partition tiles per core
RO = R * O                # 8192
F32 = mybir.dt.float32
BF16 = mybir.dt.bfloat16
AF = mybir.ActivationFunctionType

_CACHE = {}


# packed-consts layout (bf16, [128, 512]):
#   [:, 0:256]    Bm    as [128, 2, 128]  (d-chunks x r)
#   [:, 256:321]  W2p aug: row r -> [W2p[r, :], 1.0]  (bias folded, ones col
#                 makes the out2 matmul also produce s = sum_r e)
#   [:, 384:512]  K2 in row 0 only
PK = 512


def _build():
    nc = bacc.Bacc("TRN2", target_bir_lowering=False, debug=False)
    XT = nc.dram_tensor("XT", [D, BL], BF16, kind="ExternalInput")
    CP = nc.dram_tensor("CP", [128, PK], BF16, kind="ExternalInput")
    Wt = nc.dram_tensor("Wt", [D, RO], BF16, kind="ExternalInput")
    out = nc.dram_tensor("out", [BL, O], F32, kind="ExternalOutput")

    with tile.TileContext(nc) as tc, ExitStack() as ctx:
        consts = ctx.enter_context(tc.tile_pool(name="consts", bufs=1))
        mp = ctx.enter_context(tc.tile_pool(name="mp", bufs=2))
        gep = ctx.enter_context(tc.tile_pool(name="gep", bufs=3))
        gmp = ctx.enter_context(tc.tile_pool(name="gmp", bufs=2))
        trp = ctx.enter_context(tc.tile_pool(name="trp", bufs=2))
        ps_g = ctx.enter_context(tc.tile_pool(name="ps_g", bufs=2, space="PSUM"))

        # ---------------- loads ----------------
        # scalar queue: packed consts (one transfer, lands fast)
        xb_t = [consts.tile([128, 2, 128], BF16, tag=f"xb{t}", name=f"xb{t}")
                for t in range(NT)]
        cp_sb = consts.tile([128, PK], BF16, tag="cp")
        nc.sync.dma_start(out=cp_sb, in_=CP[:, :])
        bm_sb = cp_sb[:, 0:256].rearrange("p (c r) -> p c r", c=2)
        w2p_sb = cp_sb[:, 256:321]
        k2_sb = cp_sb[0:1, 384:512]
        # sync queue: XT tile 0, then Wt q0 pair, then XT tiles 1-3
        def load_xt(t, eng):
            for c in range(2):
                eng.dma_start(
                    out=xb_t[t][:, c, :],
                    in_=XT[c * 128:(c + 1) * 128, t * 128:(t + 1) * 128])

        load_xt(0, nc.sync)
        wt_q = [consts.tile([128, 2, 2048], BF16, tag=f"wt{q}", name=f"wt{q}") for q in range(4)]

        def load_wt(q, eng):
            for c in range(2):
                eng.dma_start(
                    out=wt_q[q][:, c, :],
                    in_=Wt[c * 128:(c + 1) * 128, q * 2048:(q + 1) * 2048])

        load_wt(0, nc.sync)
        for t in range(1, NT):
            load_xt(t, nc.scalar)
        for q in range(1, 4):
            load_wt(q, nc.gpsimd)

        ones_sb = consts.tile([1, 128], BF16, tag="ones")
        nc.vector.memset(ones_sb, 1.0)
        identF = consts.tile([128, 128], F32, tag="idf")
        make_identity(nc, identF)

        e_bf = [None] * NT
        rs_ = [None] * NT
        out2f = [None] * NT
        gm_t = [None] * NT
        red_t = [None] * NT

        def m_phase(t):
            # sigma is uniform, so the x^2 logit term is a per-row constant
            # that cancels in the softmax: m' = X @ Bm + K2 only.
            sm = ps_g.tile([128, 2048], F32, tag="g", name=f"sm_{t}")
            m_ps = sm[:, 0:128]
            nc.tensor.matmul(m_ps, lhsT=xb_t[t][:, 0, :], rhs=bm_sb[:, 0, :],
                             start=True, stop=False)
            nc.tensor.matmul(m_ps, lhsT=xb_t[t][:, 1, :], rhs=bm_sb[:, 1, :],
                             start=False, stop=False)
            nc.tensor.matmul(m_ps, lhsT=ones_sb, rhs=k2_sb,
                             start=False, stop=True)
            # e = exp(m')  (m' in [-1, 0], exp well-conditioned)
            ef = mp.tile([128, 128], F32, tag="ef", name=f"ef_{t}")
            nc.scalar.activation(ef, m_ps, AF.Exp)
            eb = consts.tile([128, 128], BF16, tag=f"eb{t}")
            nc.scalar.copy(eb, ef)
            # out2aug = e @ [W2p | 1]: col 64 is s = sum_r e
            eT_ps = sm[:, 128:256]
            nc.tensor.transpose(eT_ps, ef, identF)
            eTb = mp.tile([128, 128], BF16, tag="eTb", name=f"eTb_{t}")
            nc.scalar.copy(eTb, eT_ps)
            o2_ps = sm[:, 256:321]
            nc.tensor.matmul(o2_ps, lhsT=eTb, rhs=w2p_sb, start=True, stop=True)
            rs = consts.tile([128, 1], F32, tag=f"rs{t}")
            nc.vector.reciprocal(rs, o2_ps[:, 64:65])
            o2 = consts.tile([128, O], F32, tag=f"o2{t}")
            nc.scalar.copy(o2, o2_ps[:, 0:64])
            e_bf[t] = eb
            rs_[t] = rs
            out2f[t] = o2

        def tree_part(t, o0, ow, red):
            # reduce r (innermost) for o-range [o0, o0+ow): halvings to 8,
            # then one segmented reduce.
            prev = gm_t[t][:, o0:o0 + ow, :]
            sz = 64
            while sz >= 8:
                nxt = trp.tile([128, ow, sz], BF16, tag=f"tr{ow}_{sz}",
                               name=f"tr_{t}_{o0}_{sz}")
                nc.vector.tensor_add(nxt, prev[:, :, 0:sz], prev[:, :, sz:2 * sz])
                prev = nxt
                sz //= 2
            nc.vector.reduce_sum(
                red[:, o0:o0 + ow].rearrange("p o -> p o ()"),
                prev, axis=mybir.AxisListType.X)

        def finish(t, red):
            osb = mp.tile([128, O], F32, tag="osb", name=f"osb_{t}")
            nc.vector.tensor_add(osb, red, out2f[t])
            nc.vector.tensor_scalar_mul(osb, osb, rs_[t])
            nc.sync.dma_start(out=out[t * 128:(t + 1) * 128, :], in_=osb)

        ge_t = [None] * NT

        def g_block(t, blk, sub=1):
            # one psum block of 2048 cols (16 o x 128 r) = Wt quarter blk:
            # 8 matmuls, one 2048-wide ACT evict into the tile's ge buffer.
            # sub=1: per-block DVE mul; sub=2: 1024-granular (tail); sub=0:
            # defer the mul (caller issues one full-tile mul).
            if blk == 0:
                gm_t[t] = gmp.tile([128, 64, 128], BF16, tag="gm",
                                   name=f"gm_{t}")
                ge_t[t] = gep.tile([128, 8192], BF16, tag="ge",
                                   name=f"ge_{t}")
            gt = ps_g.tile([128, 2048], F32, tag="g", name=f"g_{t}_{blk}")
            for n in range(4):
                for c in range(2):
                    nc.tensor.matmul(
                        gt[:, n * 512:(n + 1) * 512],
                        lhsT=xb_t[t][:, c, :],
                        rhs=wt_q[blk][:, c, n * 512:(n + 1) * 512],
                        start=(c == 0), stop=(c == 1))
            ge = ge_t[t]
            if sub == 0:
                nc.scalar.copy(ge[:, blk * 2048:(blk + 1) * 2048], gt)
                return
            for s in range(sub):
                w = 2048 // sub
                nc.scalar.copy(ge[:, blk * 2048 + s * w:blk * 2048 + (s + 1) * w],
                               gt[:, s * w:(s + 1) * w])
                mul_w = 16 // sub
                nc.vector.tensor_mul(
                    gm_t[t][:, blk * 16 + s * mul_w:blk * 16 + (s + 1) * mul_w, :],
                    ge[:, blk * 2048 + s * w:blk * 2048 + (s + 1) * w]
                    .rearrange("p (o r) -> p o r", o=mul_w),
                    e_bf[t].rearrange("p r -> p () r").broadcast_to(
                        (128, mul_w, 128)))

        def mul_tile(t):
            nc.vector.tensor_mul(
                gm_t[t],
                ge_t[t].rearrange("p (o r) -> p o r", o=64),
                e_bf[t].rearrange("p r -> p () r").broadcast_to((128, 64, 128)))

        def g_blocks(t):
            for blk in range(4):
                g_block(t, blk)

        def tree(t):
            red_t[t] = mp.tile([128, 64], F32, tag="red", name=f"red_{t}")
            tree_part(t, 0, 64, red_t[t])
            finish(t, red_t[t])

        m_phase(0)
        m_phase(1)
        m_phase(2)
        m_phase(3)
        for blk in range(4):
            g_block(0, blk, sub=0)
            g_block(1, blk, sub=0)
        mul_tile(0)
        mul_tile(1)
        tree(0)
        tree(1)
        for blk in range(3):
            g_block(2, blk)
            g_block(3, blk)
            if blk == 1:
                red_t[2] = mp.tile([128, 64], F32, tag="red", name="red_2")
                tree_part(2, 0, 32, red_t[2])
                red_t[3] = mp.tile([128, 64], F32, tag="red", name="red_3")
                tree_part(3, 0, 32, red_t[3])
        g_block(2, 3)
        tree_part(2, 32, 32, red_t[2])
        finish(2, red_t[2])
        tree_part(3, 32, 16, red_t[3])
        g_block(3, 3, sub=2)
        tree_part(3, 48, 16, red_t[3])
        finish(3, red_t[3])

    nc.finalize()
    return nc


def _get_nc():
    if "nc" not in _CACHE:
        _CACHE["nc"] = _build()
    return _CACHE["nc"]


def _host_prep(centers, sigmas, W, b):
    c64 = centers.astype(np.float64)
    S = (H / sigmas.astype(np.float64) ** 2) + EPS          # (D,R)
    Bm = (2.0 * S * c64 / D).astype(ml_dtypes.bfloat16)      # X coeff
    K2 = (-(S * c64 * c64).sum(axis=0, keepdims=True) / D).astype(
        ml_dtypes.bfloat16
    )
    W1 = W[: D * R].reshape(R, D, O)
    # (o,r) column layout: Wt[d, o*R + r] = W1[r, d, o]
    Wt = np.ascontiguousarray(W1.transpose(1, 2, 0).reshape(D, RO)).astype(
        ml_dtypes.bfloat16
    )
    W2p = (W[D * R:].astype(np.float64) + b[None, :].astype(np.float64)).astype(
        ml_dtypes.bfloat16
    )
    CP = np.zeros((128, PK), dtype=ml_dtypes.bfloat16)
    CP[:, 0:128] = Bm[0:128]
    CP[:, 128:256] = Bm[128:256]
    CP[:, 256:320] = W2p
    CP[:, 320] = 1.0
    CP[0, 384:512] = K2[0]
    return CP, Wt


def _numpy_fallback(X, centers, sigmas, W, b):
    scale = H / sigmas.astype(np.float64) ** 2 + EPS
    d = -(X[:, :, None].astype(np.float64) - centers[None].astype(np.float64)) ** 2 * scale
    m = d.mean(axis=1)
    e = np.exp(m - m.max(1, keepdims=True))
    frs = e / e.sum(1, keepdims=True)
    xp = (X[:, None, :].astype(np.float64) * frs[:, :, None]).reshape(X.shape[0], -1)
    xp = np.concatenate([xp, frs], axis=1)
    return (xp @ W.astype(np.float64) + b).astype(np.float32)


def kernel(X, centers, sigmas, W, b):
    X = np.asarray(X, dtype=np.float32)
    centers = np.asarray(centers, dtype=np.float32)
    sigmas = np.asarray(sigmas, dtype=np.float32)
    W = np.asarray(W, dtype=np.float32)
    b = np.asarray(b, dtype=np.float32)

    if np.ptp(sigmas) != 0:
        # non-uniform sigma: the x^2 logit term no longer cancels; use the
        # (never hit in practice) reference fallback
        return _numpy_fallback(X, centers, sigmas, W, b)
    CP, Wt = _host_prep(centers, sigmas, W, b)
    nc = _get_nc()
    XTfull = np.ascontiguousarray(X.T).astype(ml_dtypes.bfloat16)  # (D, B)
    in_maps = [
        {
            "XT": np.ascontiguousarray(XTfull[:, k * BL:(k + 1) * BL]),
            "CP": CP, "Wt": Wt,
        }
        for k in range(NCORES)
    ]
    res = bass_utils.run_bass_kernel_spmd(nc, in_maps, core_ids=list(range(NCORES)))
    return np.concatenate([res.results[k]["out"] for k in range(NCORES)], axis=0)



# revision 22
# speedup vs baseline: 1.0556x; 1.0556x over previous
"""HTSK fuzzy-system kernel for Trainium2 (Bass/Tile), 8-core data-parallel.

Math (per batch row b):
  S     = H/sigma^2 + EPS                          (D,R)
  m     = mean_d(-(X_bd - C_dr)^2 * S_dr)          (B,R)
        = X^2 @ (-S/D) + X @ (2*S*C/D) + K2        (matmul expansion)
  e     = exp(m)            (m <= 0 always, so no max-subtraction needed)
  out   = (1/sum_r e) * ( sum_r e_br * G_bro  +  e @ (W2 + 1 b^T) )
  G     = X @ Wt,  Wt[d, o*R + r] = W[r*D+d, o]    (B, O*R)  "(o,r)" col layout

The (o,r) column layout makes the e-broadcast multiply a stride-1-innermost
DVE op (2x mode) and the r-reduction a halving tree over the innermost axis.

Sharding: batch B=4096 split 512 rows per core; weights replicated.
"""
import sys
import types
from contextlib import ExitStack

import numpy as np

sys.path.insert(0, "/opt/trn_rl_repo")

# NTFF profile-hook registry: trn_boot sets it at jax init, concourse
# bass_utils reads it when trace=True. The container's antenv package lacks
# this submodule, so provide it before anything imports jax/concourse.
if "antenv.axon_hooks" not in sys.modules:
    _ah = types.ModuleType("antenv.axon_hooks")
    _ah._hook = None

    def _set_hook(hook):
        _ah._hook = hook

    def _get_hook():
        return _ah._hook

    _ah.set_axon_ntff_profile_hook = _set_hook
    _ah.get_axon_ntff_profile_hook = _get_hook
    sys.modules["antenv.axon_hooks"] = _ah

import ml_dtypes  # noqa: E402
import concourse.bass as bass  # noqa: E402
import concourse.bacc as bacc  # noqa: E402
import concourse.tile as tile  # noqa: E402
from concourse import mybir  # noqa: E402
from concourse import bass_utils  # noqa: E402
from concourse.masks import make_identity  # noqa: E402

H = 0.5
EPS = 1e-8
B, D, R, O = 4096, 256, 128, 64
NCORES = 8
BL = B // NCORES          # 512 batch rows per core
NT = BL // 128            # 4 partition tiles per core
RO = R * O                # 8192
F32 = mybir.dt.float32
BF16 = mybir.dt.bfloat16
AF = mybir.ActivationFunctionType

_CACHE = {}


# packed-consts layout (bf16, [128, 512]):
#   [:, 0:256]    Bm    as [128, 2, 128]  (d-chunks x r)
#   [:, 256:321]  W2p aug: row r -> [W2p[r, :], 1.0]  (bias folded, ones col
#                 makes the out2 matmul also produce s = sum_r e)
#   [:, 384:512]  K2 in row 0 only
PK = 512


def _build():
    nc = bacc.Bacc("TRN2", target_bir_lowering=False, debug=False)
    XT = nc.dram_tensor("XT", [D, BL], BF16, kind="ExternalInput")
    CP = nc.dram_tensor("CP", [128, PK], BF16, kind="ExternalInput")
    Wt = nc.dram_tensor("Wt", [D, RO], BF16, kind="ExternalInput")
    out = nc.dram_tensor("out", [BL, O], F32, kind="ExternalOutput")

    with tile.TileContext(nc) as tc, ExitStack() as ctx:
        consts = ctx.enter_context(tc.tile_pool(name="consts", bufs=1))
        mp = ctx.enter_context(tc.tile_pool(name="mp", bufs=2))
        gep = ctx.enter_context(tc.tile_pool(name="gep", bufs=3))
        gmp = ctx.enter_context(tc.tile_pool(name="gmp", bufs=2))
        trp = ctx.enter_context(tc.tile_pool(name="trp", bufs=2))
        ps_g = ctx.enter_context(tc.tile_pool(name="ps_g", bufs=2, space="PSUM"))

        # ---------------- loads ----------------
        # scalar queue: packed consts (one transfer, lands fast)
        xb_t = [consts.tile([128, 2, 128], BF16, tag=f"xb{t}", name=f"xb{t}")
                for t in range(NT)]
        cp_sb = consts.tile([128, PK], BF16, tag="cp")
        nc.scalar.dma_start(out=cp_sb, in_=CP[:, :])
        bm_sb = cp_sb[:, 0:256].rearrange("p (c r) -> p c r", c=2)
        w2p_sb = cp_sb[:, 256:321]
        k2_sb = cp_sb[0:1, 384:512]
        # sync queue: XT tile 0, then Wt q0 pair, then XT tiles 1-3
        def load_xt(t, eng):
            for c in range(2):
                eng.dma_start(
                    out=xb_t[t][:, c, :],
                    in_=XT[c * 128:(c + 1) * 128, t * 128:(t + 1) * 128])

        load_xt(0, nc.sync)
        wt_q = [consts.tile([128, 2, 2048], BF16, tag=f"wt{q}", name=f"wt{q}") for q in range(4)]

        def load_wt(q, eng):
            for c in range(2):
                eng.dma_start(
                    out=wt_q[q][:, c, :],
                    in_=Wt[c * 128:(c + 1) * 128, q * 2048:(q + 1) * 2048])

        load_wt(0, nc.sync)
        for t in range(1, NT):
            load_xt(t, nc.scalar)
        for q in range(1, 4):
            load_wt(q, nc.gpsimd)

        ones_sb = consts.tile([1, 128], BF16, tag="ones")
        nc.vector.memset(ones_sb, 1.0)
        identF = consts.tile([128, 128], F32, tag="idf")
        make_identity(nc, identF)

        e_bf = [None] * NT
        rs_ = [None] * NT
        out2f = [None] * NT
        gm_t = [None] * NT
        red_t = [None] * NT

        def m_phase(t):
            # sigma is uniform, so the x^2 logit term is a per-row constant
            # that cancels in the softmax: m' = X @ Bm + K2 only.
            sm = ps_g.tile([128, 2048], F32, tag="g", name=f"sm_{t}")
            m_ps = sm[:, 0:128]
            nc.tensor.matmul(m_ps, lhsT=xb_t[t][:, 0, :], rhs=bm_sb[:, 0, :],
                             start=True, stop=False)
            nc.tensor.matmul(m_ps, lhsT=xb_t[t][:, 1, :], rhs=bm_sb[:, 1, :],
                             start=False, stop=False)
            nc.tensor.matmul(m_ps, lhsT=ones_sb, rhs=k2_sb,
                             start=False, stop=True)
            # e = exp(m')  (m' in [-1, 0], exp well-conditioned)
            ef = mp.tile([128, 128], F32, tag="ef", name=f"ef_{t}")
            nc.scalar.activation(ef, m_ps, AF.Exp)
            eb = consts.tile([128, 128], BF16, tag=f"eb{t}")
            nc.scalar.copy(eb, ef)
            # out2aug = e @ [W2p | 1]: col 64 is s = sum_r e
            eT_ps = sm[:, 128:256]
            nc.tensor.transpose(eT_ps, ef, identF)
            eTb = mp.tile([128, 128], BF16, tag="eTb", name=f"eTb_{t}")
            nc.scalar.copy(eTb, eT_ps)
            o2_ps = sm[:, 256:321]
            nc.tensor.matmul(o2_ps, lhsT=eTb, rhs=w2p_sb, start=True, stop=True)
            rs = consts.tile([128, 1], F32, tag=f"rs{t}")
            nc.vector.reciprocal(rs, o2_ps[:, 64:65])
            o2 = consts.tile([128, O], F32, tag=f"o2{t}")
            nc.scalar.copy(o2, o2_ps[:, 0:64])
            e_bf[t] = eb
            rs_[t] = rs
            out2f[t] = o2

        def tree_part(t, o0, ow, red):
            # reduce r (innermost) for o-range [o0, o0+ow): halvings to 8,
            # then one segmented reduce.
            prev = gm_t[t][:, o0:o0 + ow, :]
            sz = 64
            while sz >= 8:
                nxt = trp.tile([128, ow, sz], BF16, tag=f"tr{ow}_{sz}",
                               name=f"tr_{t}_{o0}_{sz}")
                nc.vector.tensor_add(nxt, prev[:, :, 0:sz], prev[:, :, sz:2 * sz])
                prev = nxt
                sz //= 2
            nc.vector.reduce_sum(
                red[:, o0:o0 + ow].rearrange("p o -> p o ()"),
                prev, axis=mybir.AxisListType.X)

        def finish(t, red):
            osb = mp.tile([128, O], F32, tag="osb", name=f"osb_{t}")
            nc.vector.tensor_add(osb, red, out2f[t])
            nc.vector.tensor_scalar_mul(osb, osb, rs_[t])
            nc.sync.dma_start(out=out[t * 128:(t + 1) * 128, :], in_=osb)

        ge_t = [None] * NT

        def g_block(t, blk, sub=1):
            # one psum block of 2048 cols (16 o x 128 r) = Wt quarter blk:
            # 8 matmuls, one 2048-wide ACT evict into the tile's ge buffer.
            # sub=1: per-block DVE mul; sub=2: 1024-granular (tail); sub=0:
            # defer the mul (caller issues one full-tile mul).
            if blk == 0:
                gm_t[t] = gmp.tile([128, 64, 128], BF16, tag="gm",
                                   name=f"gm_{t}")
                ge_t[t] = gep.tile([128, 8192], BF16, tag="ge",
                                   name=f"ge_{t}")
            gt = ps_g.tile([128, 2048], F32, tag="g", name=f"g_{t}_{blk}")
            for n in range(4):
                for c in range(2):
                    nc.tensor.matmul(
                        gt[:, n * 512:(n + 1) * 512],
                        lhsT=xb_t[t][:, c, :],
                        rhs=wt_q[blk][:, c, n * 512:(n + 1) * 512],
                        start=(c == 0), stop=(c == 1))
            ge = ge_t[t]
            if sub == 0:
                nc.scalar.copy(ge[:, blk * 2048:(blk + 1) * 2048], gt)
                return
            for s in range(sub):
                w = 2048 // sub
                nc.scalar.copy(ge[:, blk * 2048 + s * w:blk * 2048 + (s + 1) * w],
                               gt[:, s * w:(s + 1) * w])
                mul_w = 16 // sub
                nc.vector.tensor_mul(
                    gm_t[t][:, blk * 16 + s * mul_w:blk * 16 + (s + 1) * mul_w, :],
                    ge[:, blk * 2048 + s * w:blk * 2048 + (s + 1) * w]
                    .rearrange("p (o r) -> p o r", o=mul_w),
                    e_bf[t].rearrange("p r -> p () r").broadcast_to(
                        (128, mul_w, 128)))

        def mul_tile(t):
            nc.vector.tensor_mul(
                gm_t[t],
                ge_t[t].rearrange("p (o r) -> p o r", o=64),
                e_bf[t].rearrange("p r -> p () r").broadcast_to((128, 64, 128)))

        def g_blocks(t):
            for blk in range(4):
                g_block(t, blk)

        def tree(t):
            red_t[t] = mp.tile([128, 64], F32, tag="red", name=f"red_{t}")
            tree_part(t, 0, 64, red_t[t])
            finish(t, red_t[t])

        m_phase(0)
        m_phase(1)
        m_phase(2)
        m_phase(3)
        for blk in range(4):
            g_block(0, blk)
            g_block(1, blk)
        tree(0)
        tree(1)
        for blk in range(3):
            g_block(2, blk)
            g_block(3, blk)
            if blk == 1:
                red_t[2] = mp.tile([128, 64], F32, tag="red", name="red_2")
                tree_part(2, 0, 32, red_t[2])
                red_t[3] = mp.tile([128, 64], F32, tag="red", name="red_3")
                tree_part(3, 0, 32, red_t[3])
        g_block(2, 3)
        tree_part(2, 32, 32, red_t[2])
        finish(2, red_t[2])
        tree_part(3, 32, 16, red_t[3])
        g_block(3, 3, sub=2)
        tree_part(3, 48, 16, red_t[3])
        finish(3, red_t[3])

    nc.finalize()
    return nc


def _get_nc():
    if "nc" not in _CACHE:
        _CACHE["nc"] = _build()
    return _CACHE["nc"]


def _host_prep(centers, sigmas, W, b):
    c64 = centers.astype(np.float64)
    S = (H / sigmas.astype(np.float64) ** 2) + EPS          # (D,R)
    Bm = (2.0 * S * c64 / D).astype(ml_dtypes.bfloat16)      # X coeff
    K2 = (-(S * c64 * c64).sum(axis=0, keepdims=True) / D).astype(
        ml_dtypes.bfloat16
    )
    W1 = W[: D * R].reshape(R, D, O)
    # (o,r) column layout: Wt[d, o*R + r] = W1[r, d, o]
    Wt = np.ascontiguousarray(W1.transpose(1, 2, 0).reshape(D, RO)).astype(
        ml_dtypes.bfloat16
    )
    W2p = (W[D * R:].astype(np.float64) + b[None, :].astype(np.float64)).astype(
        ml_dtypes.bfloat16
    )
    CP = np.zeros((128, PK), dtype=ml_dtypes.bfloat16)
    CP[:, 0:128] = Bm[0:128]
    CP[:, 128:256] = Bm[128:256]
    CP[:, 256:320] = W2p
    CP[:, 320] = 1.0
    CP[0, 384:512] = K2[0]
    return CP, Wt


def _numpy_fallback(X, centers, sigmas, W, b):
    scale = H / sigmas.astype(np.float64) ** 2 + EPS
    d = -(X[:, :, None].astype(np.float64) - centers[None].astype(np.float64)) ** 2 * scale
    m = d.mean(axis=1)
    e = np.exp(m - m.max(1, keepdims=True))
    frs = e / e.sum(1, keepdims=True)
    xp = (X[:, None, :].astype(np.float64) * frs[:, :, None]).reshape(X.shape[0], -1)
    xp = np.concatenate([xp, frs], axis=1)
    return (xp @ W.astype(np.float64) + b).astype(np.float32)


def kernel(X, centers, sigmas, W, b):
    X = np.asarray(X, dtype=np.float32)
    centers = np.asarray(centers, dtype=np.float32)
    sigmas = np.asarray(sigmas, dtype=np.float32)
    W = np.asarray(W, dtype=np.float32)
    b = np.asarray(b, dtype=np.float32)

    if np.ptp(sigmas) != 0:
        # non-uniform sigma: the x^2 logit term no longer cancels; use the
        # (never hit in practice) reference fallback
        return _numpy_fallback(X, centers, sigmas, W, b)
    CP, Wt = _host_prep(centers, sigmas, W, b)
    nc = _get_nc()
    XTfull = np.ascontiguousarray(X.T).astype(ml_dtypes.bfloat16)  # (D, B)
    in_maps = [
        {
            "XT": np.ascontiguousarray(XTfull[:, k * BL:(k + 1) * BL]),
            "CP": CP, "Wt": Wt,
        }
        for k in range(NCORES)
    ]
    res = bass_utils.run_bass_kernel_spmd(nc, in_maps, core_ids=list(range(NCORES)))
    return np.concatenate([res.results[k]["out"] for k in range(NCORES)], axis=0)



# revision 23
# speedup vs baseline: 1.1061x; 1.0479x over previous
"""HTSK fuzzy-system kernel for Trainium2 (Bass/Tile), 8-core data-parallel.

Math (per batch row b):
  S     = H/sigma^2 + EPS                          (D,R)
  m     = mean_d(-(X_bd - C_dr)^2 * S_dr)          (B,R)
        = X^2 @ (-S/D) + X @ (2*S*C/D) + K2        (matmul expansion)
  e     = exp(m)            (m <= 0 always, so no max-subtraction needed)
  out   = (1/sum_r e) * ( sum_r e_br * G_bro  +  e @ (W2 + 1 b^T) )
  G     = X @ Wt,  Wt[d, o*R + r] = W[r*D+d, o]    (B, O*R)  "(o,r)" col layout

The (o,r) column layout makes the e-broadcast multiply a stride-1-innermost
DVE op (2x mode) and the r-reduction a halving tree over the innermost axis.

Sharding: batch B=4096 split 512 rows per core; weights replicated.
"""
import sys
import types
from contextlib import ExitStack

import numpy as np

sys.path.insert(0, "/opt/trn_rl_repo")

# NTFF profile-hook registry: trn_boot sets it at jax init, concourse
# bass_utils reads it when trace=True. The container's antenv package lacks
# this submodule, so provide it before anything imports jax/concourse.
if "antenv.axon_hooks" not in sys.modules:
    _ah = types.ModuleType("antenv.axon_hooks")
    _ah._hook = None

    def _set_hook(hook):
        _ah._hook = hook

    def _get_hook():
        return _ah._hook

    _ah.set_axon_ntff_profile_hook = _set_hook
    _ah.get_axon_ntff_profile_hook = _get_hook
    sys.modules["antenv.axon_hooks"] = _ah

import ml_dtypes  # noqa: E402
import concourse.bass as bass  # noqa: E402
import concourse.bacc as bacc  # noqa: E402
import concourse.tile as tile  # noqa: E402
from concourse import mybir  # noqa: E402
from concourse import bass_utils  # noqa: E402
from concourse.masks import make_identity  # noqa: E402

H = 0.5
EPS = 1e-8
B, D, R, O = 4096, 256, 128, 64
NCORES = 8
BL = B // NCORES          # 512 batch rows per core
NT = BL // 128            # 4 partition tiles per core
RO = R * O                # 8192
F32 = mybir.dt.float32
BF16 = mybir.dt.bfloat16
AF = mybir.ActivationFunctionType

_CACHE = {}


# packed-consts layout (bf16, [128, 512]):
#   [:, 0:256]    Bm    as [128, 2, 128]  (d-chunks x r)
#   [:, 256:321]  W2p aug: row r -> [W2p[r, :], 1.0]  (bias folded, ones col
#                 makes the out2 matmul also produce s = sum_r e)
#   [:, 384:512]  K2 in row 0 only
PK = 512


def _build():
    nc = bacc.Bacc("TRN2", target_bir_lowering=False, debug=False)
    XT = nc.dram_tensor("XT", [D, BL], BF16, kind="ExternalInput")
    CP = nc.dram_tensor("CP", [128, PK], BF16, kind="ExternalInput")
    Wt = nc.dram_tensor("Wt", [D, RO], BF16, kind="ExternalInput")
    out = nc.dram_tensor("out", [BL, O], F32, kind="ExternalOutput")

    with tile.TileContext(nc) as tc, ExitStack() as ctx:
        consts = ctx.enter_context(tc.tile_pool(name="consts", bufs=1))
        mp = ctx.enter_context(tc.tile_pool(name="mp", bufs=2))
        gep = ctx.enter_context(tc.tile_pool(name="gep", bufs=3))
        gmp = ctx.enter_context(tc.tile_pool(name="gmp", bufs=2))
        trp = ctx.enter_context(tc.tile_pool(name="trp", bufs=2))
        ps_g = ctx.enter_context(tc.tile_pool(name="ps_g", bufs=2, space="PSUM"))

        # ---------------- loads ----------------
        # scalar queue: packed consts (one transfer, lands fast)
        xb_t = [consts.tile([128, 2, 128], BF16, tag=f"xb{t}", name=f"xb{t}")
                for t in range(NT)]
        cp_sb = consts.tile([128, PK], BF16, tag="cp")
        nc.scalar.dma_start(out=cp_sb, in_=CP[:, :])
        bm_sb = cp_sb[:, 0:256].rearrange("p (c r) -> p c r", c=2)
        w2p_sb = cp_sb[:, 256:321]
        k2_sb = cp_sb[0:1, 384:512]
        # sync queue: XT tile 0, then Wt q0 pair, then XT tiles 1-3
        def load_xt(t, eng):
            for c in range(2):
                eng.dma_start(
                    out=xb_t[t][:, c, :],
                    in_=XT[c * 128:(c + 1) * 128, t * 128:(t + 1) * 128])

        load_xt(0, nc.sync)
        wt_q = [consts.tile([128, 2, 2048], BF16, tag=f"wt{q}", name=f"wt{q}") for q in range(4)]

        def load_wt(q, eng):
            for c in range(2):
                eng.dma_start(
                    out=wt_q[q][:, c, :],
                    in_=Wt[c * 128:(c + 1) * 128, q * 2048:(q + 1) * 2048])

        load_wt(0, nc.sync)
        for t in range(1, NT):
            load_xt(t, nc.scalar)
        for q in range(1, 4):
            load_wt(q, nc.gpsimd)

        ones_sb = consts.tile([1, 128], BF16, tag="ones")
        nc.vector.memset(ones_sb, 1.0)
        identF = consts.tile([128, 128], F32, tag="idf")
        make_identity(nc, identF)

        e_bf = [None] * NT
        rs_ = [None] * NT
        out2f = [None] * NT
        gm_t = [None] * NT
        red_t = [None] * NT

        def m_phase(t):
            # sigma is uniform, so the x^2 logit term is a per-row constant
            # that cancels in the softmax: m' = X @ Bm + K2 only.
            sm = ps_g.tile([128, 2048], F32, tag="g", name=f"sm_{t}")
            m_ps = sm[:, 0:128]
            nc.tensor.matmul(m_ps, lhsT=xb_t[t][:, 0, :], rhs=bm_sb[:, 0, :],
                             start=True, stop=False)
            nc.tensor.matmul(m_ps, lhsT=xb_t[t][:, 1, :], rhs=bm_sb[:, 1, :],
                             start=False, stop=False)
            nc.tensor.matmul(m_ps, lhsT=ones_sb, rhs=k2_sb,
                             start=False, stop=True)
            # e = exp(m')  (m' in [-1, 0], exp well-conditioned)
            ef = mp.tile([128, 128], F32, tag="ef", name=f"ef_{t}")
            nc.scalar.activation(ef, m_ps, AF.Exp)
            eb = consts.tile([128, 128], BF16, tag=f"eb{t}")
            nc.scalar.copy(eb, ef)
            # out2aug = e @ [W2p | 1]: col 64 is s = sum_r e
            eT_ps = sm[:, 128:256]
            nc.tensor.transpose(eT_ps, ef, identF)
            eTb = mp.tile([128, 128], BF16, tag="eTb", name=f"eTb_{t}")
            nc.scalar.copy(eTb, eT_ps)
            o2_ps = sm[:, 256:321]
            nc.tensor.matmul(o2_ps, lhsT=eTb, rhs=w2p_sb, start=True, stop=True)
            rs = consts.tile([128, 1], F32, tag=f"rs{t}")
            nc.vector.reciprocal(rs, o2_ps[:, 64:65])
            o2 = consts.tile([128, O], F32, tag=f"o2{t}")
            nc.scalar.copy(o2, o2_ps[:, 0:64])
            e_bf[t] = eb
            rs_[t] = rs
            out2f[t] = o2

        def tree_part(t, o0, ow, red):
            # reduce r (innermost) for o-range [o0, o0+ow): halvings to 8,
            # then one segmented reduce.
            prev = gm_t[t][:, o0:o0 + ow, :]
            sz = 64
            while sz >= 8:
                nxt = trp.tile([128, ow, sz], BF16, tag=f"tr{ow}_{sz}",
                               name=f"tr_{t}_{o0}_{sz}")
                nc.vector.tensor_add(nxt, prev[:, :, 0:sz], prev[:, :, sz:2 * sz])
                prev = nxt
                sz //= 2
            nc.vector.reduce_sum(
                red[:, o0:o0 + ow].rearrange("p o -> p o ()"),
                prev, axis=mybir.AxisListType.X)

        def finish(t, red):
            osb = mp.tile([128, O], F32, tag="osb", name=f"osb_{t}")
            nc.vector.tensor_add(osb, red, out2f[t])
            nc.vector.tensor_scalar_mul(osb, osb, rs_[t])
            nc.sync.dma_start(out=out[t * 128:(t + 1) * 128, :], in_=osb)

        ge_t = [None] * NT

        def g_block(t, blk, sub=1):
            # one psum block of 2048 cols (16 o x 128 r) = Wt quarter blk:
            # 8 matmuls, one 2048-wide ACT evict into the tile's ge buffer.
            # sub=1: per-block DVE mul; sub=2: 1024-granular (tail); sub=0:
            # defer the mul (caller issues one full-tile mul).
            if blk == 0:
                gm_t[t] = gmp.tile([128, 64, 128], BF16, tag="gm",
                                   name=f"gm_{t}")
                ge_t[t] = gep.tile([128, 8192], BF16, tag="ge",
                                   name=f"ge_{t}")
            gt = ps_g.tile([128, 2048], F32, tag="g", name=f"g_{t}_{blk}")
            for n in range(4):
                for c in range(2):
                    nc.tensor.matmul(
                        gt[:, n * 512:(n + 1) * 512],
                        lhsT=xb_t[t][:, c, :],
                        rhs=wt_q[blk][:, c, n * 512:(n + 1) * 512],
                        start=(c == 0), stop=(c == 1))
            ge = ge_t[t]
            if sub == 0:
                nc.scalar.copy(ge[:, blk * 2048:(blk + 1) * 2048], gt)
                return
            for s in range(sub):
                w = 2048 // sub
                nc.scalar.copy(ge[:, blk * 2048 + s * w:blk * 2048 + (s + 1) * w],
                               gt[:, s * w:(s + 1) * w])
                mul_w = 16 // sub
                nc.vector.tensor_mul(
                    gm_t[t][:, blk * 16 + s * mul_w:blk * 16 + (s + 1) * mul_w, :],
                    ge[:, blk * 2048 + s * w:blk * 2048 + (s + 1) * w]
                    .rearrange("p (o r) -> p o r", o=mul_w),
                    e_bf[t].rearrange("p r -> p () r").broadcast_to(
                        (128, mul_w, 128)))

        def mul_tile(t):
            nc.vector.tensor_mul(
                gm_t[t],
                ge_t[t].rearrange("p (o r) -> p o r", o=64),
                e_bf[t].rearrange("p r -> p () r").broadcast_to((128, 64, 128)))

        def g_blocks(t):
            for blk in range(4):
                g_block(t, blk)

        def tree(t):
            red_t[t] = mp.tile([128, 64], F32, tag="red", name=f"red_{t}")
            tree_part(t, 0, 64, red_t[t])
            finish(t, red_t[t])

        m_phase(0)
        m_phase(1)
        m_phase(2)
        m_phase(3)
        for blk in range(4):
            g_block(0, blk)
            g_block(1, blk)
            if blk == 1:
                red_t[0] = mp.tile([128, 64], F32, tag="red", name="red_0")
                tree_part(0, 0, 32, red_t[0])
                red_t[1] = mp.tile([128, 64], F32, tag="red", name="red_1")
                tree_part(1, 0, 32, red_t[1])
        tree_part(0, 32, 32, red_t[0])
        finish(0, red_t[0])
        tree_part(1, 32, 32, red_t[1])
        finish(1, red_t[1])
        for blk in range(3):
            g_block(2, blk)
            g_block(3, blk)
            if blk == 1:
                red_t[2] = mp.tile([128, 64], F32, tag="red", name="red_2")
                tree_part(2, 0, 32, red_t[2])
                red_t[3] = mp.tile([128, 64], F32, tag="red", name="red_3")
                tree_part(3, 0, 32, red_t[3])
        g_block(2, 3)
        tree_part(2, 32, 32, red_t[2])
        finish(2, red_t[2])
        tree_part(3, 32, 16, red_t[3])
        g_block(3, 3, sub=2)
        tree_part(3, 48, 16, red_t[3])
        finish(3, red_t[3])

    nc.finalize()
    return nc


def _get_nc():
    if "nc" not in _CACHE:
        _CACHE["nc"] = _build()
    return _CACHE["nc"]


def _host_prep(centers, sigmas, W, b):
    c64 = centers.astype(np.float64)
    S = (H / sigmas.astype(np.float64) ** 2) + EPS          # (D,R)
    Bm = (2.0 * S * c64 / D).astype(ml_dtypes.bfloat16)      # X coeff
    K2 = (-(S * c64 * c64).sum(axis=0, keepdims=True) / D).astype(
        ml_dtypes.bfloat16
    )
    W1 = W[: D * R].reshape(R, D, O)
    # (o,r) column layout: Wt[d, o*R + r] = W1[r, d, o]
    Wt = np.ascontiguousarray(W1.transpose(1, 2, 0).reshape(D, RO)).astype(
        ml_dtypes.bfloat16
    )
    W2p = (W[D * R:].astype(np.float64) + b[None, :].astype(np.float64)).astype(
        ml_dtypes.bfloat16
    )
    CP = np.zeros((128, PK), dtype=ml_dtypes.bfloat16)
    CP[:, 0:128] = Bm[0:128]
    CP[:, 128:256] = Bm[128:256]
    CP[:, 256:320] = W2p
    CP[:, 320] = 1.0
    CP[0, 384:512] = K2[0]
    return CP, Wt


def _numpy_fallback(X, centers, sigmas, W, b):
    scale = H / sigmas.astype(np.float64) ** 2 + EPS
    d = -(X[:, :, None].astype(np.float64) - centers[None].astype(np.float64)) ** 2 * scale
    m = d.mean(axis=1)
    e = np.exp(m - m.max(1, keepdims=True))
    frs = e / e.sum(1, keepdims=True)
    xp = (X[:, None, :].astype(np.float64) * frs[:, :, None]).reshape(X.shape[0], -1)
    xp = np.concatenate([xp, frs], axis=1)
    return (xp @ W.astype(np.float64) + b).astype(np.float32)


def kernel(X, centers, sigmas, W, b):
    X = np.asarray(X, dtype=np.float32)
    centers = np.asarray(centers, dtype=np.float32)
    sigmas = np.asarray(sigmas, dtype=np.float32)
    W = np.asarray(W, dtype=np.float32)
    b = np.asarray(b, dtype=np.float32)

    if np.ptp(sigmas) != 0:
        # non-uniform sigma: the x^2 logit term no longer cancels; use the
        # (never hit in practice) reference fallback
        return _numpy_fallback(X, centers, sigmas, W, b)
    CP, Wt = _host_prep(centers, sigmas, W, b)
    nc = _get_nc()
    XTfull = np.ascontiguousarray(X.T).astype(ml_dtypes.bfloat16)  # (D, B)
    in_maps = [
        {
            "XT": np.ascontiguousarray(XTfull[:, k * BL:(k + 1) * BL]),
            "CP": CP, "Wt": Wt,
        }
        for k in range(NCORES)
    ]
    res = bass_utils.run_bass_kernel_spmd(nc, in_maps, core_ids=list(range(NCORES)))
    return np.concatenate([res.results[k]["out"] for k in range(NCORES)], axis=0)

